# revision 5
# baseline (speedup 1.0000x reference)
"""Bahdanau attention forward on 8 Trainium2 NeuronCores.

reference:
    qh     = h_t @ W_h.T                     [B, D]
    kh     = keys @ W_k.T                    [B, N, D]
    energy = tanh(qh[:, None, :] + kh)       [B, N, D]
    scores = energy @ v                      [B, N]
    alpha  = softmax(scores, -1)             [B, N]
    context= alpha @ keys                    [B, D]
    return (context, alpha)

Sharding: data-parallel over batch B=64 across 8 cores (8 batches/core);
weights replicated. No cross-core communication.

Per-core device pipeline (all matmuls bf16 with fp32 PSUM accumulation):
  - keys batch slab -> SBUF natural layout via SWDGE cast-DMA (fp32->bf16)
  - keysT via 64 xbar DMA transposes (or PE-transpose fallback)
  - khT[e, n] = W_kT.T @ keysT, per 128-row e-tile in PSUM
  - energyT = tanh(khT + qh) on ScalarE with per-partition bias = qhT[:, b]
  - scores[1, n] += v_et.T @ energyT_et  (v-as-weights matmuls)
  - softmax on [1, N] (DVE reduce + ACT exp with accumulated sum)
  - alphaT[n, 1] per n-tile via K=1 matmul against ones (PE transpose of alpha)
  - context[1, d] += alphaT_nt.T @ keys_nat_nt
"""

import os
import numpy as np
import ml_dtypes

B, N, D = 64, 1024, 1024
NCORES = 8
B_LOC = B // NCORES
P = 128
ET = D // P
DT = D // P
NT = N // P
NH = N // 512  # 512-wide psum column halves

USE_XBAR_TRANSPOSE = os.environ.get("BAHDANAU_PE_TRANSPOSE", "0") != "1"

_compiled = None


def _emit(nc, tc, ctx, aps):
    import concourse.mybir as mybir

    f32 = mybir.dt.float32
    bf16 = mybir.dt.bfloat16
    Tanh = mybir.ActivationFunctionType.Tanh
    Exp = mybir.ActivationFunctionType.Exp
    X = mybir.AxisListType.X

    keys_l, wkT, whT, htT, v_col, ctx_out, alpha_out = aps

    consts = ctx.enter_context(tc.tile_pool(name="consts", bufs=1))
    knat_pool = ctx.enter_context(tc.tile_pool(name="knat", bufs=2))
    kT_pool = ctx.enter_context(tc.tile_pool(name="kT", bufs=2))
    en_pool = ctx.enter_context(tc.tile_pool(name="energy", bufs=3))
    sm_pool = ctx.enter_context(tc.tile_pool(name="sm", bufs=2))
    psum_kh = ctx.enter_context(tc.tile_pool(name="psum_kh", bufs=2, space="PSUM"))
    psum_misc = ctx.enter_context(tc.tile_pool(name="psum_misc", bufs=2, space="PSUM"))
    if not USE_XBAR_TRANSPOSE:
        psum_tr = ctx.enter_context(tc.tile_pool(name="psum_tr", bufs=2, space="PSUM"))
        ident = consts.tile([P, P], bf16)

    wkT_sb = consts.tile([P, DT, D], bf16)
    nc.sync.dma_start(out=wkT_sb[:], in_=wkT.rearrange("(dt p) e -> p dt e", p=P))
    whT_sb = consts.tile([P, DT, D], bf16)
    nc.sync.dma_start(out=whT_sb[:], in_=whT.rearrange("(dt p) e -> p dt e", p=P))
    htT_sb = consts.tile([P, DT, B_LOC], bf16)
    nc.sync.dma_start(out=htT_sb[:], in_=htT.rearrange("(dt p) b -> p dt b", p=P))
    v_sb = consts.tile([P, ET], bf16)
    nc.sync.dma_start(out=v_sb[:], in_=v_col)
    ones_f32 = consts.tile([1, 1], f32)
    nc.vector.memset(ones_f32[:], 1.0)
    if not USE_XBAR_TRANSPOSE:
        from concourse.masks import make_identity

        make_identity(nc, ident[:])

    # qhT[e-tile, b] = (h_t @ W_h.T).T, once per core
    qhT_sb = consts.tile([P, ET, B_LOC], f32)
    for et in range(ET):
        pq = psum_misc.tile([P, B_LOC], f32, tag="misc")
        for dt in range(DT):
            nc.tensor.matmul(
                pq[:],
                whT_sb[:, dt, et * P : (et + 1) * P],
                htT_sb[:, dt, :],
                start=(dt == 0),
                stop=(dt == DT - 1),
            )
        nc.vector.tensor_copy(out=qhT_sb[:, et, :], in_=pq[:])

    for b in range(B_LOC):
        # load keys batch slab, casting fp32 -> bf16 in flight
        knat = knat_pool.tile([P, NT, D], bf16, tag="knat")
        nc.gpsimd.dma_start(
            out=knat[:], in_=keys_l[b].rearrange("(nt p) d -> p nt d", p=P)
        )

        # transpose to keysT blocks: kT[:, dt*NT+nt, :] = keys[nt-tile, dt-cols].T
        kT = kT_pool.tile([P, DT * NT, P], bf16, tag="kT")
        if USE_XBAR_TRANSPOSE:
            for dt in range(DT):
                for nt in range(NT):
                    nc.sync.dma_start(
                        out=kT[:, dt * NT + nt, :],
                        in_=knat[:, nt, dt * P : (dt + 1) * P],
                        transpose=True,
                    )
        else:
            for dt in range(DT):
                for half in range(2):
                    pt = psum_tr.tile([P, 4 * P], bf16, tag="tr")
                    for k in range(4):
                        nt = half * 4 + k
                        nc.tensor.transpose(
                            pt[:, k * P : (k + 1) * P],
                            knat[:, nt, dt * P : (dt + 1) * P],
                            ident[:],
                        )
                    nc.vector.tensor_copy(
                        out=kT[:, dt * NT + half * 4 : dt * NT + (half + 1) * 4, :],
                        in_=pt[:],
                    )

        # scores accumulators [1, 512] x2
        sc = [psum_misc.tile([1, 512], f32, tag="misc", name=f"sc{i}") for i in range(NH)]
        for et in range(ET):
            pk = psum_kh.tile([P, N], f32, tag="kh")
            for dt in range(DT):
                lhsT = wkT_sb[:, dt, et * P : (et + 1) * P]
                for nh in range(NH):
                    nc.tensor.matmul(
                        pk[:, nh * 512 : (nh + 1) * 512],
                        lhsT,
                        kT[:, dt * NT + nh * 4 : dt * NT + (nh + 1) * 4, :],
                        start=(dt == 0),
                        stop=(dt == DT - 1),
                    )
            en = en_pool.tile([P, N], bf16, tag="en")
            nc.scalar.activation(
                out=en[:],
                in_=pk[:],
                func=Tanh,
                bias=qhT_sb[:, et, b : b + 1],
                scale=1.0,
            )
            for nh in range(NH):
                nc.tensor.matmul(
                    sc[nh][:],
                    v_sb[:, et : et + 1],
                    en[:, nh * 512 : (nh + 1) * 512],
                    start=(et == 0),
                    stop=(et == ET - 1),
                )

        # softmax over [1, N]
        sc_sb = sm_pool.tile([1, N], f32, tag="sc_sb")
        for nh in range(NH):
            nc.vector.tensor_copy(out=sc_sb[:, nh * 512 : (nh + 1) * 512], in_=sc[nh][:])
        nmx = sm_pool.tile([1, 1], f32, tag="nmx")
        nc.vector.tensor_reduce(
            nmx[:], sc_sb[:], axis=X, op=mybir.AluOpType.max, negate=True
        )
        ex = sm_pool.tile([1, N], f32, tag="ex")
        ssum = sm_pool.tile([1, 1], f32, tag="ssum")
        nc.scalar.activation(
            out=ex[:], in_=sc_sb[:], func=Exp, bias=nmx[:], scale=1.0, accum_out=ssum[:]
        )
        rcp = sm_pool.tile([1, 1], f32, tag="rcp")
        nc.vector.reciprocal(rcp[:], ssum[:])
        alpha_sb = sm_pool.tile([1, N], f32, tag="alpha_sb")
        nc.vector.tensor_scalar_mul(alpha_sb[:], ex[:], rcp[:])
        nc.sync.dma_start(out=alpha_out[b : b + 1, :], in_=alpha_sb[:])

        # alphaT[n, 1] per n-tile: K=1 matmul against ones
        pat = psum_misc.tile([P, NT], f32, tag="misc")
        for nt in range(NT):
            nc.tensor.matmul(
                pat[:, nt : nt + 1],
                alpha_sb[0:1, nt * P : (nt + 1) * P],
                ones_f32[:],
                start=True,
                stop=True,
            )
        alphaT_sb = sm_pool.tile([P, NT], bf16, tag="alphaT")
        nc.vector.tensor_copy(out=alphaT_sb[:], in_=pat[:])

        # context[1, d] = sum_nt alphaT_nt.T @ keys_nat_nt
        cx = [psum_misc.tile([1, 512], f32, tag="misc", name=f"cx{i}") for i in range(NH)]
        for nt in range(NT):
            for nh in range(NH):
                nc.tensor.matmul(
                    cx[nh][:],
                    alphaT_sb[:, nt : nt + 1],
                    knat[:, nt, nh * 512 : (nh + 1) * 512],
                    start=(nt == 0),
                    stop=(nt == NT - 1),
                )
        ctx_sb = sm_pool.tile([1, D], f32, tag="ctx_sb")
        for nh in range(NH):
            nc.vector.tensor_copy(out=ctx_sb[:, nh * 512 : (nh + 1) * 512], in_=cx[nh][:])
        nc.sync.dma_start(out=ctx_out[b : b + 1, :], in_=ctx_sb[:])


def _build():
    from contextlib import ExitStack

    import concourse.mybir as mybir
    import concourse.tile as tile
    from concourse import bacc

    f32 = mybir.dt.float32
    bf16 = mybir.dt.bfloat16

    nc = bacc.Bacc("TRN2", target_bir_lowering=False, debug=False, num_devices=NCORES)
    keys_l = nc.dram_tensor("keys_l", [B_LOC, N, D], f32, kind="ExternalInput")
    wkT = nc.dram_tensor("wkT", [D, D], bf16, kind="ExternalInput")
    whT = nc.dram_tensor("whT", [D, D], bf16, kind="ExternalInput")
    htT = nc.dram_tensor("htT", [D, B_LOC], bf16, kind="ExternalInput")
    v_col = nc.dram_tensor("v_col", [P, ET], bf16, kind="ExternalInput")
    ctx_out = nc.dram_tensor("ctx_out", [B_LOC, D], f32, kind="ExternalOutput")
    alpha_out = nc.dram_tensor("alpha_out", [B_LOC, N], f32, kind="ExternalOutput")

    aps = (
        keys_l.ap(),
        wkT.ap(),
        whT.ap(),
        htT.ap(),
        v_col.ap(),
        ctx_out.ap(),
        alpha_out.ap(),
    )
    with tile.TileContext(nc) as tc:
        with ExitStack() as ctx:
            _emit(nc, tc, ctx, aps)
    nc.compile()
    return nc


def _get_compiled():
    global _compiled
    if _compiled is None:
        _compiled = _build()
    return _compiled


def _install_prof_shim():
    """Shim antenv.axon_hooks so run_bass_kernel_spmd(trace=True) can
    NTFF-profile under axon; neuter the bucket artifact upload."""
    import sys
    import types

    if "antenv.axon_hooks" not in sys.modules:
        import antenv

        mod = types.ModuleType("antenv.axon_hooks")
        mod._hook = None
        mod.set_axon_ntff_profile_hook = lambda h: setattr(mod, "_hook", h)
        mod.get_axon_ntff_profile_hook = lambda: mod._hook
        sys.modules["antenv.axon_hooks"] = mod
        antenv.axon_hooks = mod
        try:
            from trn_agent_boot.trn_boot import _ntff_profile_via_ctypes

            mod._hook = _ntff_profile_via_ctypes("/opt/axon/libaxon_pjrt.so")
        except Exception:
            pass

    from concourse import bass_utils

    bass_utils.upload_artifacts = lambda tmpdir: f"local://{tmpdir}"


def kernel(h_t, keys, W_h, W_k, v):
    from concourse import bass_utils

    h_t = np.asarray(h_t, dtype=np.float32)
    keys = np.ascontiguousarray(np.asarray(keys, dtype=np.float32))
    W_h = np.asarray(W_h, dtype=np.float32)
    W_k = np.asarray(W_k, dtype=np.float32)
    v = np.asarray(v, dtype=np.float32)

    bf = ml_dtypes.bfloat16
    wkT = np.ascontiguousarray(W_k.T).astype(bf)
    whT = np.ascontiguousarray(W_h.T).astype(bf)
    v_col = np.ascontiguousarray(v.reshape(ET, P).T).astype(bf)

    in_maps = []
    for c in range(NCORES):
        sl = slice(c * B_LOC, (c + 1) * B_LOC)
        in_maps.append(
            {
                "keys_l": keys[sl],
                "wkT": wkT,
                "whT": whT,
                "htT": np.ascontiguousarray(h_t[sl].T).astype(bf),
                "v_col": v_col,
            }
        )

    nc = _get_compiled()

    trace = os.environ.get("BAHDANAU_TRACE", "0") == "1"
    if trace:
        _install_prof_shim()
    res = bass_utils.run_bass_kernel_spmd(
        nc, in_maps, core_ids=list(range(NCORES)), trace=trace
    )
    if trace:
        kernel.last_exec_time_ns = res.exec_time_ns
        kernel.last_results = res

    context = np.concatenate([res.results[c]["ctx_out"] for c in range(NCORES)], axis=0)
    alpha = np.concatenate([res.results[c]["alpha_out"] for c in range(NCORES)], axis=0)
    return (context, alpha)


# revision 8
# speedup vs baseline: 2.2550x; 2.2550x over previous
"""Bahdanau attention forward on 8 Trainium2 NeuronCores.

reference:
    qh     = h_t @ W_h.T                     [B, D]
    kh     = keys @ W_k.T                    [B, N, D]
    energy = tanh(qh[:, None, :] + kh)       [B, N, D]
    scores = energy @ v                      [B, N]
    alpha  = softmax(scores, -1)             [B, N]
    context= alpha @ keys                    [B, D]
    return (context, alpha)

Sharding: data-parallel over batch B=64 across 8 cores (8 batches/core);
weights replicated. No cross-core communication.

Per-core device pipeline (all matmuls bf16 with fp32 PSUM accumulation):
  - keys batch slab -> SBUF natural layout via SWDGE cast-DMA (fp32->bf16)
  - keysT via 64 xbar DMA transposes (or PE-transpose fallback)
  - khT[e, n] = W_kT.T @ keysT, per 128-row e-tile in PSUM
  - energyT = tanh(khT + qh) on ScalarE with per-partition bias = qhT[:, b]
  - scores[1, n] += v_et.T @ energyT_et  (v-as-weights matmuls)
  - softmax on [1, N] (DVE reduce + ACT exp with accumulated sum)
  - alphaT[n, 1] per n-tile via K=1 matmul against ones (PE transpose of alpha)
  - context[1, d] += alphaT_nt.T @ keys_nat_nt
"""

import os
import numpy as np
import ml_dtypes

B, N, D = 64, 1024, 1024
NCORES = 8
B_LOC = B // NCORES
P = 128
ET = D // P
DT = D // P
NT = N // P
NH = N // 512  # 512-wide psum column halves

USE_XBAR_TRANSPOSE = os.environ.get("BAHDANAU_PE_TRANSPOSE", "0") != "1"

_compiled = None


def _emit(nc, tc, ctx, aps):
    import concourse.mybir as mybir

    f32 = mybir.dt.float32
    bf16 = mybir.dt.bfloat16
    Tanh = mybir.ActivationFunctionType.Tanh
    Exp = mybir.ActivationFunctionType.Exp
    X = mybir.AxisListType.X

    keys_l, wkT, whT, htT, v_col, ctx_out, alpha_out = aps

    consts = ctx.enter_context(tc.tile_pool(name="consts", bufs=1))
    knat_pool = ctx.enter_context(tc.tile_pool(name="knat", bufs=2))
    kT_pool = ctx.enter_context(tc.tile_pool(name="kT", bufs=2))
    en_pool = ctx.enter_context(tc.tile_pool(name="energy", bufs=3))
    sm_pool = ctx.enter_context(tc.tile_pool(name="sm", bufs=2))
    psum_kh = ctx.enter_context(tc.tile_pool(name="psum_kh", bufs=2, space="PSUM"))
    psum_misc = ctx.enter_context(tc.tile_pool(name="psum_misc", bufs=2, space="PSUM"))
    if not USE_XBAR_TRANSPOSE:
        psum_tr = ctx.enter_context(tc.tile_pool(name="psum_tr", bufs=2, space="PSUM"))
        ident = consts.tile([P, P], bf16)

    wkT_sb = consts.tile([P, DT, D], bf16)
    nc.sync.dma_start(out=wkT_sb[:], in_=wkT.rearrange("(dt p) e -> p dt e", p=P))
    whT_sb = consts.tile([P, DT, D], bf16)
    nc.sync.dma_start(out=whT_sb[:], in_=whT.rearrange("(dt p) e -> p dt e", p=P))
    htT_sb = consts.tile([P, DT, B_LOC], bf16)
    nc.sync.dma_start(out=htT_sb[:], in_=htT.rearrange("(dt p) b -> p dt b", p=P))
    v_sb = consts.tile([P, ET], bf16)
    nc.sync.dma_start(out=v_sb[:], in_=v_col)
    ones_f32 = consts.tile([1, 1], f32)
    nc.vector.memset(ones_f32[:], 1.0)
    if not USE_XBAR_TRANSPOSE:
        from concourse.masks import make_identity

        make_identity(nc, ident[:])

    # qhT[e-tile, b] = (h_t @ W_h.T).T, once per core
    qhT_sb = consts.tile([P, ET, B_LOC], f32)
    for et in range(ET):
        pq = psum_misc.tile([P, B_LOC], f32, tag="misc")
        for dt in range(DT):
            nc.tensor.matmul(
                pq[:],
                whT_sb[:, dt, et * P : (et + 1) * P],
                htT_sb[:, dt, :],
                start=(dt == 0),
                stop=(dt == DT - 1),
            )
        nc.vector.tensor_copy(out=qhT_sb[:, et, :], in_=pq[:])

    for b in range(B_LOC):
        # load keys batch slab, casting fp32 -> bf16 in flight
        knat = knat_pool.tile([P, NT, D], bf16, tag="knat")
        nc.gpsimd.dma_start(
            out=knat[:], in_=keys_l[b].rearrange("(nt p) d -> p nt d", p=P)
        )

        # transpose to keysT blocks.
        # xbar path: one [128, 8192] -> [128, 64, 128] transpose; result slab
        # s = nt*DT + dt holds keys[nt-tile, dt-cols].T (kTr below reindexes).
        kT = kT_pool.tile([P, DT * NT, P], bf16, tag="kT")
        if USE_XBAR_TRANSPOSE:
            nc.sync.dma_start(out=kT[:], in_=knat[:], transpose=True)
            kTr = kT.rearrange("p (nt dt) c -> p nt dt c", dt=DT)
        else:
            for dt in range(DT):
                for half in range(2):
                    pt = psum_tr.tile([P, 4 * P], bf16, tag="tr")
                    for k in range(4):
                        nt = half * 4 + k
                        nc.tensor.transpose(
                            pt[:, k * P : (k + 1) * P],
                            knat[:, nt, dt * P : (dt + 1) * P],
                            ident[:],
                        )
                    nc.vector.tensor_copy(
                        out=kT[:, dt * NT + half * 4 : dt * NT + (half + 1) * 4, :],
                        in_=pt[:],
                    )

        def kh_rhs(dt, nh):
            if USE_XBAR_TRANSPOSE:
                return kTr[:, nh * 4 : (nh + 1) * 4, dt, :]
            return kT[:, dt * NT + nh * 4 : dt * NT + (nh + 1) * 4, :]

        # scores accumulators [1, 512] x2
        sc = [psum_misc.tile([1, 512], f32, tag="misc", name=f"sc{i}") for i in range(NH)]
        for et in range(ET):
            pk = psum_kh.tile([P, N], f32, tag="kh")
            for dt in range(DT):
                lhsT = wkT_sb[:, dt, et * P : (et + 1) * P]
                for nh in range(NH):
                    nc.tensor.matmul(
                        pk[:, nh * 512 : (nh + 1) * 512],
                        lhsT,
                        kh_rhs(dt, nh),
                        start=(dt == 0),
                        stop=(dt == DT - 1),
                    )
            en = en_pool.tile([P, N], bf16, tag="en")
            nc.scalar.activation(
                out=en[:],
                in_=pk[:],
                func=Tanh,
                bias=qhT_sb[:, et, b : b + 1],
                scale=1.0,
            )
            for nh in range(NH):
                nc.tensor.matmul(
                    sc[nh][:],
                    v_sb[:, et : et + 1],
                    en[:, nh * 512 : (nh + 1) * 512],
                    start=(et == 0),
                    stop=(et == ET - 1),
                )

        # softmax over [1, N]
        sc_sb = sm_pool.tile([1, N], f32, tag="sc_sb")
        for nh in range(NH):
            nc.vector.tensor_copy(out=sc_sb[:, nh * 512 : (nh + 1) * 512], in_=sc[nh][:])
        nmx = sm_pool.tile([1, 1], f32, tag="nmx")
        nc.vector.tensor_reduce(
            nmx[:], sc_sb[:], axis=X, op=mybir.AluOpType.max, negate=True
        )
        ex = sm_pool.tile([1, N], f32, tag="ex")
        ssum = sm_pool.tile([1, 1], f32, tag="ssum")
        nc.scalar.activation(
            out=ex[:], in_=sc_sb[:], func=Exp, bias=nmx[:], scale=1.0, accum_out=ssum[:]
        )
        rcp = sm_pool.tile([1, 1], f32, tag="rcp")
        nc.vector.reciprocal(rcp[:], ssum[:])
        alpha_sb = sm_pool.tile([1, N], f32, tag="alpha_sb")
        nc.vector.tensor_scalar_mul(alpha_sb[:], ex[:], rcp[:])
        nc.sync.dma_start(out=alpha_out[b : b + 1, :], in_=alpha_sb[:])

        # alphaT[n, 1] per n-tile: K=1 matmul against ones
        pat = psum_misc.tile([P, NT], f32, tag="misc")
        for nt in range(NT):
            nc.tensor.matmul(
                pat[:, nt : nt + 1],
                alpha_sb[0:1, nt * P : (nt + 1) * P],
                ones_f32[:],
                start=True,
                stop=True,
            )
        alphaT_sb = sm_pool.tile([P, NT], bf16, tag="alphaT")
        nc.vector.tensor_copy(out=alphaT_sb[:], in_=pat[:])

        # context[1, d] = sum_nt alphaT_nt.T @ keys_nat_nt
        cx = [psum_misc.tile([1, 512], f32, tag="misc", name=f"cx{i}") for i in range(NH)]
        for nt in range(NT):
            for nh in range(NH):
                nc.tensor.matmul(
                    cx[nh][:],
                    alphaT_sb[:, nt : nt + 1],
                    knat[:, nt, nh * 512 : (nh + 1) * 512],
                    start=(nt == 0),
                    stop=(nt == NT - 1),
                )
        ctx_sb = sm_pool.tile([1, D], f32, tag="ctx_sb")
        for nh in range(NH):
            nc.vector.tensor_copy(out=ctx_sb[:, nh * 512 : (nh + 1) * 512], in_=cx[nh][:])
        nc.sync.dma_start(out=ctx_out[b : b + 1, :], in_=ctx_sb[:])


def _build():
    from contextlib import ExitStack

    import concourse.mybir as mybir
    import concourse.tile as tile
    from concourse import bacc

    f32 = mybir.dt.float32
    bf16 = mybir.dt.bfloat16

    nc = bacc.Bacc("TRN2", target_bir_lowering=False, debug=False, num_devices=NCORES)
    keys_l = nc.dram_tensor("keys_l", [B_LOC, N, D], f32, kind="ExternalInput")
    wkT = nc.dram_tensor("wkT", [D, D], bf16, kind="ExternalInput")
    whT = nc.dram_tensor("whT", [D, D], bf16, kind="ExternalInput")
    htT = nc.dram_tensor("htT", [D, B_LOC], bf16, kind="ExternalInput")
    v_col = nc.dram_tensor("v_col", [P, ET], bf16, kind="ExternalInput")
    ctx_out = nc.dram_tensor("ctx_out", [B_LOC, D], f32, kind="ExternalOutput")
    alpha_out = nc.dram_tensor("alpha_out", [B_LOC, N], f32, kind="ExternalOutput")

    aps = (
        keys_l.ap(),
        wkT.ap(),
        whT.ap(),
        htT.ap(),
        v_col.ap(),
        ctx_out.ap(),
        alpha_out.ap(),
    )
    with tile.TileContext(nc) as tc:
        with ExitStack() as ctx:
            _emit(nc, tc, ctx, aps)
    nc.compile()
    return nc


def _get_compiled():
    global _compiled
    if _compiled is None:
        _compiled = _build()
    return _compiled


def _install_prof_shim():
    """Shim antenv.axon_hooks so run_bass_kernel_spmd(trace=True) can
    NTFF-profile under axon; neuter the bucket artifact upload."""
    import sys
    import types

    if "antenv.axon_hooks" not in sys.modules:
        import antenv

        mod = types.ModuleType("antenv.axon_hooks")
        mod._hook = None
        mod.set_axon_ntff_profile_hook = lambda h: setattr(mod, "_hook", h)
        mod.get_axon_ntff_profile_hook = lambda: mod._hook
        sys.modules["antenv.axon_hooks"] = mod
        antenv.axon_hooks = mod
        try:
            from trn_agent_boot.trn_boot import _ntff_profile_via_ctypes

            mod._hook = _ntff_profile_via_ctypes("/opt/axon/libaxon_pjrt.so")
        except Exception:
            pass

    from concourse import bass_utils

    bass_utils.upload_artifacts = lambda tmpdir: f"local://{tmpdir}"


def kernel(h_t, keys, W_h, W_k, v):
    from concourse import bass_utils

    h_t = np.asarray(h_t, dtype=np.float32)
    keys = np.ascontiguousarray(np.asarray(keys, dtype=np.float32))
    W_h = np.asarray(W_h, dtype=np.float32)
    W_k = np.asarray(W_k, dtype=np.float32)
    v = np.asarray(v, dtype=np.float32)

    bf = ml_dtypes.bfloat16
    wkT = np.ascontiguousarray(W_k.T).astype(bf)
    whT = np.ascontiguousarray(W_h.T).astype(bf)
    v_col = np.ascontiguousarray(v.reshape(ET, P).T).astype(bf)

    in_maps = []
    for c in range(NCORES):
        sl = slice(c * B_LOC, (c + 1) * B_LOC)
        in_maps.append(
            {
                "keys_l": keys[sl],
                "wkT": wkT,
                "whT": whT,
                "htT": np.ascontiguousarray(h_t[sl].T).astype(bf),
                "v_col": v_col,
            }
        )

    nc = _get_compiled()

    trace = os.environ.get("BAHDANAU_TRACE", "0") == "1"
    if trace:
        _install_prof_shim()
    res = bass_utils.run_bass_kernel_spmd(
        nc, in_maps, core_ids=list(range(NCORES)), trace=trace
    )
    if trace:
        kernel.last_exec_time_ns = res.exec_time_ns
        kernel.last_results = res

    context = np.concatenate([res.results[c]["ctx_out"] for c in range(NCORES)], axis=0)
    alpha = np.concatenate([res.results[c]["alpha_out"] for c in range(NCORES)], axis=0)
    return (context, alpha)


# revision 12
# speedup vs baseline: 2.4254x; 1.0756x over previous
"""Bahdanau attention forward on 8 Trainium2 NeuronCores.

reference:
    qh     = h_t @ W_h.T                     [B, D]
    kh     = keys @ W_k.T                    [B, N, D]
    energy = tanh(qh[:, None, :] + kh)       [B, N, D]
    scores = energy @ v                      [B, N]
    alpha  = softmax(scores, -1)             [B, N]
    context= alpha @ keys                    [B, D]
    return (context, alpha)

Sharding: data-parallel over batch B=64 across 8 cores (8 batches/core);
weights replicated. No cross-core communication.

Per-core device pipeline (all matmuls bf16 with fp32 PSUM accumulation):
  - keys batch slab -> SBUF natural layout via SWDGE cast-DMA (fp32->bf16)
  - keysT via 64 xbar DMA transposes (or PE-transpose fallback)
  - khT[e, n] = W_kT.T @ keysT, per 128-row e-tile in PSUM
  - energyT = tanh(khT + qh) on ScalarE with per-partition bias = qhT[:, b]
  - scores[1, n] += v_et.T @ energyT_et  (v-as-weights matmuls)
  - softmax on [1, N] (DVE reduce + ACT exp with accumulated sum)
  - alphaT[n, 1] per n-tile via K=1 matmul against ones (PE transpose of alpha)
  - context[1, d] += alphaT_nt.T @ keys_nat_nt
"""

import os
import numpy as np
import ml_dtypes

B, N, D = 64, 1024, 1024
NCORES = 8
B_LOC = B // NCORES
P = 128
ET = D // P
DT = D // P
NT = N // P
NH = N // 512  # 512-wide psum column halves

USE_XBAR_TRANSPOSE = os.environ.get("BAHDANAU_PE_TRANSPOSE", "0") != "1"

_compiled = None


def _emit(nc, tc, ctx, aps):
    import concourse.mybir as mybir

    f32 = mybir.dt.float32
    bf16 = mybir.dt.bfloat16
    Tanh = mybir.ActivationFunctionType.Tanh
    Exp = mybir.ActivationFunctionType.Exp
    X = mybir.AxisListType.X

    keys_l, wkT, whT, htT, v_col, ctx_out, alpha_out = aps

    consts = ctx.enter_context(tc.tile_pool(name="consts", bufs=1))
    knat_pool = ctx.enter_context(tc.tile_pool(name="knat", bufs=4))
    kT_pool = ctx.enter_context(tc.tile_pool(name="kT", bufs=3))
    en_pool = ctx.enter_context(tc.tile_pool(name="energy", bufs=3))
    sm_pool = ctx.enter_context(tc.tile_pool(name="sm", bufs=2))
    psum_kh = ctx.enter_context(tc.tile_pool(name="psum_kh", bufs=2, space="PSUM"))
    psum_misc = ctx.enter_context(tc.tile_pool(name="psum_misc", bufs=4, space="PSUM"))
    if not USE_XBAR_TRANSPOSE:
        psum_tr = ctx.enter_context(tc.tile_pool(name="psum_tr", bufs=2, space="PSUM"))
        ident = consts.tile([P, P], bf16)

    wkT_sb = consts.tile([P, DT, D], bf16)
    nc.sync.dma_start(out=wkT_sb[:], in_=wkT.rearrange("(dt p) e -> p dt e", p=P))
    whT_sb = consts.tile([P, DT, D], bf16)
    nc.sync.dma_start(out=whT_sb[:], in_=whT.rearrange("(dt p) e -> p dt e", p=P))
    htT_sb = consts.tile([P, DT, B_LOC], bf16)
    nc.sync.dma_start(out=htT_sb[:], in_=htT.rearrange("(dt p) b -> p dt b", p=P))
    v_sb = consts.tile([P, ET], bf16)
    nc.sync.dma_start(out=v_sb[:], in_=v_col)
    ones_f32 = consts.tile([1, 1], f32)
    nc.vector.memset(ones_f32[:], 1.0)
    if not USE_XBAR_TRANSPOSE:
        from concourse.masks import make_identity

        make_identity(nc, ident[:])

    # qhT[e-tile, b] = (h_t @ W_h.T).T, once per core
    qhT_sb = consts.tile([P, ET, B_LOC], f32)
    for et in range(ET):
        pq = psum_misc.tile([P, B_LOC], f32, tag="misc")
        for dt in range(DT):
            nc.tensor.matmul(
                pq[:],
                whT_sb[:, dt, et * P : (et + 1) * P],
                htT_sb[:, dt, :],
                start=(dt == 0),
                stop=(dt == DT - 1),
            )
        nc.vector.tensor_copy(out=qhT_sb[:, et, :], in_=pq[:])

    # keys load + transpose, prefetched PF batches ahead of compute
    PF = 2
    knats: dict[int, object] = {}
    kTs: dict[int, object] = {}

    def prefetch(b):
        if b >= B_LOC:
            return
        knat = knat_pool.tile([P, NT, D], bf16, tag="knat", name=f"knat{b}")
        nc.gpsimd.dma_start(
            out=knat[:], in_=keys_l[b].rearrange("(nt p) d -> p nt d", p=P)
        )
        # transpose to keysT blocks.
        # xbar path: one [128, 8192] -> [128, 64, 128] transpose; result slab
        # s = nt*DT + dt holds keys[nt-tile, dt-cols].T.
        kT = kT_pool.tile([P, DT * NT, P], bf16, tag="kT", name=f"kT{b}")
        if USE_XBAR_TRANSPOSE:
            nc.sync.dma_start(out=kT[:], in_=knat[:], transpose=True)
        else:
            for dt in range(DT):
                for half in range(2):
                    pt = psum_tr.tile([P, 4 * P], bf16, tag="tr")
                    for k in range(4):
                        nt = half * 4 + k
                        nc.tensor.transpose(
                            pt[:, k * P : (k + 1) * P],
                            knat[:, nt, dt * P : (dt + 1) * P],
                            ident[:],
                        )
                    nc.vector.tensor_copy(
                        out=kT[:, dt * NT + half * 4 : dt * NT + (half + 1) * 4, :],
                        in_=pt[:],
                    )
        knats[b] = knat
        kTs[b] = kT

    def kh_rhs(kT, dt, nh):
        if USE_XBAR_TRANSPOSE:
            kTr = kT.rearrange("p (nt dt) c -> p nt dt c", dt=DT)
            return kTr[:, nh * 4 : (nh + 1) * 4, dt, :]
        return kT[:, dt * NT + nh * 4 : dt * NT + (nh + 1) * 4, :]

    def tail_phase(b, alpha_sb):
        """alphaT + context matmuls for batch b (emitted one batch late so the
        PE can chew on batch b+1's kh matmuls while softmax_b finishes)."""
        knat = knats.pop(b)
        pat = psum_misc.tile([P, NT], f32, tag="misc", name=f"pat{b}")
        for nt in range(NT):
            nc.tensor.matmul(
                pat[:, nt : nt + 1],
                alpha_sb[0:1, nt * P : (nt + 1) * P],
                ones_f32[:],
                start=True,
                stop=True,
            )
        alphaT_sb = sm_pool.tile([P, NT], bf16, tag="alphaT", name=f"alphaT{b}")
        nc.vector.tensor_copy(out=alphaT_sb[:], in_=pat[:])
        cx = [
            psum_misc.tile([1, 512], f32, tag="misc", name=f"cx{b}_{i}")
            for i in range(NH)
        ]
        for nt in range(NT):
            for nh in range(NH):
                nc.tensor.matmul(
                    cx[nh][:],
                    alphaT_sb[:, nt : nt + 1],
                    knat[:, nt, nh * 512 : (nh + 1) * 512],
                    start=(nt == 0),
                    stop=(nt == NT - 1),
                )
        ctx_sb = sm_pool.tile([1, D], f32, tag="ctx_sb", name=f"ctx_sb{b}")
        for nh in range(NH):
            nc.vector.tensor_copy(
                out=ctx_sb[:, nh * 512 : (nh + 1) * 512], in_=cx[nh][:]
            )
        nc.sync.dma_start(out=ctx_out[b : b + 1, :], in_=ctx_sb[:])

    for b in range(min(PF, B_LOC)):
        prefetch(b)
    pending = None

    for b in range(B_LOC):
        knat = knats[b]
        kT = kTs.pop(b)

        # scores accumulators [1, 512] x2
        sc = [psum_misc.tile([1, 512], f32, tag="misc", name=f"sc{i}") for i in range(NH)]
        for et in range(ET):
            pk = psum_kh.tile([P, N], f32, tag="kh")
            for dt in range(DT):
                lhsT = wkT_sb[:, dt, et * P : (et + 1) * P]
                for nh in range(NH):
                    nc.tensor.matmul(
                        pk[:, nh * 512 : (nh + 1) * 512],
                        lhsT,
                        kh_rhs(kT, dt, nh),
                        start=(dt == 0),
                        stop=(dt == DT - 1),
                    )
            en = en_pool.tile([P, N], bf16, tag="en")
            nc.scalar.activation(
                out=en[:],
                in_=pk[:],
                func=Tanh,
                bias=qhT_sb[:, et, b : b + 1],
                scale=1.0,
            )
            for nh in range(NH):
                nc.tensor.matmul(
                    sc[nh][:],
                    v_sb[:, et : et + 1],
                    en[:, nh * 512 : (nh + 1) * 512],
                    start=(et == 0),
                    stop=(et == ET - 1),
                )

        # softmax over [1, N]
        sc_sb = sm_pool.tile([1, N], f32, tag="sc_sb")
        for nh in range(NH):
            nc.vector.tensor_copy(out=sc_sb[:, nh * 512 : (nh + 1) * 512], in_=sc[nh][:])
        nmx = sm_pool.tile([1, 1], f32, tag="nmx")
        nc.vector.tensor_reduce(
            nmx[:], sc_sb[:], axis=X, op=mybir.AluOpType.max, negate=True
        )
        ex = sm_pool.tile([1, N], f32, tag="ex")
        ssum = sm_pool.tile([1, 1], f32, tag="ssum")
        nc.scalar.activation(
            out=ex[:], in_=sc_sb[:], func=Exp, bias=nmx[:], scale=1.0, accum_out=ssum[:]
        )
        rcp = sm_pool.tile([1, 1], f32, tag="rcp")
        nc.vector.reciprocal(rcp[:], ssum[:])
        alpha_sb = sm_pool.tile([1, N], f32, tag="alpha_sb", name=f"alpha_sb{b}")
        nc.vector.tensor_scalar_mul(alpha_sb[:], ex[:], rcp[:])
        nc.sync.dma_start(out=alpha_out[b : b + 1, :], in_=alpha_sb[:])

        # batch b-1's alphaT + context matmuls land behind batch b's kh work
        if pending is not None:
            tail_phase(*pending)
        pending = (b, alpha_sb)
        prefetch(b + PF)

    tail_phase(*pending)


def _build():
    from contextlib import ExitStack

    import concourse.mybir as mybir
    import concourse.tile as tile
    from concourse import bacc

    f32 = mybir.dt.float32
    bf16 = mybir.dt.bfloat16

    nc = bacc.Bacc("TRN2", target_bir_lowering=False, debug=False, num_devices=NCORES)
    keys_l = nc.dram_tensor("keys_l", [B_LOC, N, D], f32, kind="ExternalInput")
    wkT = nc.dram_tensor("wkT", [D, D], bf16, kind="ExternalInput")
    whT = nc.dram_tensor("whT", [D, D], bf16, kind="ExternalInput")
    htT = nc.dram_tensor("htT", [D, B_LOC], bf16, kind="ExternalInput")
    v_col = nc.dram_tensor("v_col", [P, ET], bf16, kind="ExternalInput")
    ctx_out = nc.dram_tensor("ctx_out", [B_LOC, D], f32, kind="ExternalOutput")
    alpha_out = nc.dram_tensor("alpha_out", [B_LOC, N], f32, kind="ExternalOutput")

    aps = (
        keys_l.ap(),
        wkT.ap(),
        whT.ap(),
        htT.ap(),
        v_col.ap(),
        ctx_out.ap(),
        alpha_out.ap(),
    )
    with tile.TileContext(nc) as tc:
        with ExitStack() as ctx:
            _emit(nc, tc, ctx, aps)
    nc.compile()
    return nc


def _get_compiled():
    global _compiled
    if _compiled is None:
        _compiled = _build()
    return _compiled


def _install_prof_shim():
    """Shim antenv.axon_hooks so run_bass_kernel_spmd(trace=True) can
    NTFF-profile under axon; neuter the bucket artifact upload."""
    import sys
    import types

    if "antenv.axon_hooks" not in sys.modules:
        import antenv

        mod = types.ModuleType("antenv.axon_hooks")
        mod._hook = None
        mod.set_axon_ntff_profile_hook = lambda h: setattr(mod, "_hook", h)
        mod.get_axon_ntff_profile_hook = lambda: mod._hook
        sys.modules["antenv.axon_hooks"] = mod
        antenv.axon_hooks = mod
        try:
            from trn_agent_boot.trn_boot import _ntff_profile_via_ctypes

            mod._hook = _ntff_profile_via_ctypes("/opt/axon/libaxon_pjrt.so")
        except Exception:
            pass

    from concourse import bass_utils

    bass_utils.upload_artifacts = lambda tmpdir: f"local://{tmpdir}"


def kernel(h_t, keys, W_h, W_k, v):
    from concourse import bass_utils

    h_t = np.asarray(h_t, dtype=np.float32)
    keys = np.ascontiguousarray(np.asarray(keys, dtype=np.float32))
    W_h = np.asarray(W_h, dtype=np.float32)
    W_k = np.asarray(W_k, dtype=np.float32)
    v = np.asarray(v, dtype=np.float32)

    bf = ml_dtypes.bfloat16
    wkT = np.ascontiguousarray(W_k.T).astype(bf)
    whT = np.ascontiguousarray(W_h.T).astype(bf)
    v_col = np.ascontiguousarray(v.reshape(ET, P).T).astype(bf)

    in_maps = []
    for c in range(NCORES):
        sl = slice(c * B_LOC, (c + 1) * B_LOC)
        in_maps.append(
            {
                "keys_l": keys[sl],
                "wkT": wkT,
                "whT": whT,
                "htT": np.ascontiguousarray(h_t[sl].T).astype(bf),
                "v_col": v_col,
            }
        )

    nc = _get_compiled()

    trace = os.environ.get("BAHDANAU_TRACE", "0") == "1"
    if trace:
        _install_prof_shim()
    res = bass_utils.run_bass_kernel_spmd(
        nc, in_maps, core_ids=list(range(NCORES)), trace=trace
    )
    if trace:
        kernel.last_exec_time_ns = res.exec_time_ns
        kernel.last_results = res

    context = np.concatenate([res.results[c]["ctx_out"] for c in range(NCORES)], axis=0)
    alpha = np.concatenate([res.results[c]["alpha_out"] for c in range(NCORES)], axis=0)
    return (context, alpha)
